# revision 4
# baseline (speedup 1.0000x reference)
"""Causal self-attention (B=4, T=2048, D=1024, H=16) on 8 TRN2 NeuronCores.

Sharding: core i = (batch b = i//2, head-group g = i%2). Data parallel on B,
tensor parallel on heads (8 heads per group): qkv_proj columns and out_proj
rows split per head group. Each core computes a partial [D, T] output^T for
its batch; host sums the two group partials per batch, transposes, adds bias.

Per-core pipeline (all matmuls in float32r = FP22, full PE rate at N>=256):
  phase 1: x -> x^T via PE transpose; V = x@Wv (natural [t,d] + ones col);
           Q^T, K^T = (x@Wq)^T via transposed projection, bounced to DRAM.
  phase 2: per head pair p, per q-chunk qc (512), per k-tile j (128):
           S^T[k,q] = K^T.T @ Q^T (2 heads row-packed at partitions 0-63 /
           64-127); one exp over both heads' strips; causal mask-mul on
           diagonal tiles; AV: psum[65,512] += V'[k,d+1].T @ P^T accumulated
           over j -- row 64 is the softmax denominator (ones column).
           Normalize with reciprocal + gpsimd partition_broadcast.
  phase 3: out^T[f,t] = sum_p Wo_pair[d128,f].T @ O^T_pair[d128,t].
"""

import numpy as np

import concourse.bacc as bacc
import concourse.tile as tile
import concourse.mybir as mybir
from concourse import bass_utils
from concourse.bass import ts
from concourse.masks import make_identity

F32 = mybir.dt.float32
F32R = mybir.dt.float32r
EXP = mybir.ActivationFunctionType.Exp

T = 2048
TT = 16          # t tiles of 128
NP = 4           # head pairs per core
NQC = 4          # q chunks of 512
SCALE = 0.125    # 1/sqrt(64)

_CACHE = {}


def _build(CT):
    """CT = number of 128-row c-tiles in the (possibly bias-augmented) x/W."""
    nc = bacc.Bacc("TRN2", target_bir_lowering=False, debug=False)
    C = CT * 128

    xa = nc.dram_tensor("xa", [T, C], F32, kind="ExternalInput").ap()
    wq = nc.dram_tensor("wq", [C, 512], F32, kind="ExternalInput").ap()
    wk = nc.dram_tensor("wk", [C, 512], F32, kind="ExternalInput").ap()
    wv = nc.dram_tensor("wv", [C, 512], F32, kind="ExternalInput").ap()
    wo = nc.dram_tensor("wo", [512, 1024], F32, kind="ExternalInput").ap()
    dmask = nc.dram_tensor("dmask", [128, 4, 512], F32, kind="ExternalInput").ap()
    ot = nc.dram_tensor("ot", [1024, T], F32, kind="ExternalOutput").ap()

    with tile.TileContext(nc) as tc:
        with (
            tc.tile_pool(name="persist", bufs=1) as persist,
            tc.tile_pool(name="dram", bufs=1, space="DRAM") as dpool,
        ):
            vS = persist.tile([128, TT, 8, 65], F32R)     # [k128, ktile, head, d+1]
            OT = persist.tile([128, NP, T], F32R)         # [d128(2 heads), pair, t]
            dm = persist.tile([128, 4, 512], F32R)
            nc.sync.dma_start(out=dm, in_=dmask.bitcast(F32R))
            nc.vector.memset(vS[:, :, :, 64:65].bitcast(F32), 1.0)

            qTd = dpool.tile([128, NP, T], F32)           # Q^T bounce [f128, pair, t]
            kTd = dpool.tile([128, NP, T], F32)

            # ---------------- phase 1: transpose + projections ----------------
            with (
                tc.tile_pool(name="ph1", bufs=1) as ph1,
                tc.tile_pool(name="xnat", bufs=4) as xnat,
                tc.tile_pool(name="bounce", bufs=4) as bpool,
                tc.tile_pool(name="pst", bufs=2, space="PSUM") as pst,
                tc.tile_pool(name="psp", bufs=4, space="PSUM") as psp,
            ):
                ident = ph1.tile([128, 128], F32)
                make_identity(nc, ident)
                wv_sb = ph1.tile([128, CT, 512], F32R)
                nc.sync.dma_start(
                    out=wv_sb, in_=wv.rearrange("(ct P) f -> P ct f", P=128).bitcast(F32R)
                )
                wq_sb = ph1.tile([128, CT, NP, 128], F32R)
                nc.sync.dma_start(
                    out=wq_sb,
                    in_=wq.rearrange("(ct P) (np f) -> P ct np f", P=128, np=NP).bitcast(F32R),
                )
                wk_sb = ph1.tile([128, CT, NP, 128], F32R)
                nc.sync.dma_start(
                    out=wk_sb,
                    in_=wk.rearrange("(ct P) (np f) -> P ct np f", P=128, np=NP).bitcast(F32R),
                )
                xT = [ph1.tile([128, T], F32R, name=f"xT{cc}") for cc in range(CT)]

                def transpose_tt(tt):
                    for cc in range(CT):
                        xn = xnat.tile([128, 128], F32)
                        nc.sync.dma_start(
                            out=xn, in_=xa[ts(tt, 128), ts(cc, 128)]
                        )
                        pt_ = pst.tile([128, 512], F32)
                        nc.tensor.transpose(pt_[:, :128], xn, ident)
                        nc.vector.tensor_copy(out=xT[cc][:, ts(tt, 128)], in_=pt_[:, :128])

                def vproj_tt(tt):
                    ps = psp.tile([128, 512], F32)
                    for cc in range(CT):
                        nc.tensor.matmul(
                            ps,
                            lhsT=xT[cc][:, ts(tt, 128)],
                            rhs=wv_sb[:, cc, :],
                            start=(cc == 0),
                            stop=(cc == CT - 1),
                        )
                    nc.vector.tensor_copy(
                        out=vS[:, tt, :, 0:64],
                        in_=ps.rearrange("p (h d) -> p h d", h=8),
                    )

                def qkproj_tc(tc_):
                    for p in range(NP):
                        for w_sb, dst, scl in ((wq_sb, qTd, SCALE), (wk_sb, kTd, 1.0)):
                            ps = psp.tile([128, 512], F32)
                            for cc in range(CT):
                                nc.tensor.matmul(
                                    ps,
                                    lhsT=w_sb[:, cc, p, :],
                                    rhs=xT[cc][:, ts(tc_, 512)],
                                    start=(cc == 0),
                                    stop=(cc == CT - 1),
                                )
                            bo = bpool.tile([128, 512], F32)
                            nc.scalar.mul(out=bo, in_=ps, mul=scl)
                            nc.sync.dma_start(out=dst[:, p, ts(tc_, 512)], in_=bo)

                for tt in range(TT + 1):
                    if tt < TT:
                        transpose_tt(tt)
                    if tt >= 1:
                        vproj_tt(tt - 1)
                        if (tt - 1) % 4 == 3:
                            qkproj_tc((tt - 1) // 4)

            # ---------------- phase 2: attention ----------------
            with (
                tc.tile_pool(name="qk", bufs=2) as qkpool,
                tc.tile_pool(name="ptp", bufs=4) as ptpool,
                tc.tile_pool(name="rsm", bufs=4) as rpool,
                tc.tile_pool(name="rbc", bufs=4) as rbcpool,
                tc.tile_pool(name="psS", bufs=2, space="PSUM") as psS,
                tc.tile_pool(name="psAv", bufs=4, space="PSUM") as psAv,
            ):
                for p in range(NP):
                    qT = qkpool.tile([128, T], F32R, tag="qT")
                    nc.sync.dma_start(out=qT, in_=qTd[:, p, :].bitcast(F32R))
                    kT = qkpool.tile([128, T], F32R, tag="kT")
                    nc.sync.dma_start(out=kT, in_=kTd[:, p, :].bitcast(F32R))
                    for qc in range(NQC):
                        nj = 4 * qc + 4
                        av = [psAv.tile([128, 512], F32, name="av", tag="av") for _ in range(2)]
                        pts = [None] * nj

                        def s_exp(j):
                            sg = psS.tile([128, 2, 512], F32)
                            for m in range(2):
                                nc.tensor.matmul(
                                    sg[:, m, :],
                                    lhsT=kT[64 * m : 64 * m + 64, ts(j, 128)],
                                    rhs=qT[64 * m : 64 * m + 64, ts(qc, 512)],
                                    start=True,
                                    stop=True,
                                )
                            ptile = ptpool.tile([128, 2, 512], F32R)
                            nc.scalar.activation(out=ptile, in_=sg, func=EXP)
                            if j >= 4 * qc:
                                jm = j - 4 * qc
                                nc.vector.tensor_mul(
                                    ptile,
                                    ptile,
                                    dm[:, jm, None, :].to_broadcast([128, 2, 512]),
                                )
                            pts[j] = ptile

                        def av_mm(j):
                            for m in range(2):
                                nc.tensor.matmul(
                                    av[m][:65, :],
                                    lhsT=vS[:, j, 2 * p + m, :],
                                    rhs=pts[j][:, m, :],
                                    start=(j == 0),
                                    stop=(j == nj - 1),
                                )
                            pts[j] = None

                        # software pipeline: S/exp one j ahead of AV
                        for j in range(nj + 1):
                            if j < nj:
                                s_exp(j)
                            if j >= 1:
                                av_mm(j - 1)

                        for m in range(2):
                            rinv = rpool.tile([1, 512], F32)
                            nc.vector.reciprocal(out=rinv, in_=av[m][64:65, :])
                            rb = rbcpool.tile([64, 512], F32)
                            nc.gpsimd.partition_broadcast(rb, rinv)
                            nc.vector.tensor_mul(
                                OT[64 * m : 64 * m + 64, p, ts(qc, 512)],
                                av[m][0:64, :],
                                rb,
                            )

            # ---------------- phase 3: output projection ----------------
            with (
                tc.tile_pool(name="ph3", bufs=1) as ph3,
                tc.tile_pool(name="obnc", bufs=4) as opool,
                tc.tile_pool(name="psO", bufs=8, space="PSUM") as psO,
            ):
                wo_sb = ph3.tile([128, NP, 1024], F32R)
                nc.sync.dma_start(
                    out=wo_sb, in_=wo.rearrange("(np P) f -> P np f", P=128).bitcast(F32R)
                )
                for ft in range(8):
                    pso = [psO.tile([128, 512], F32, name="pso", tag="pso") for _ in range(4)]
                    for p in range(NP):
                        for tc_ in range(4):
                            nc.tensor.matmul(
                                pso[tc_],
                                lhsT=wo_sb[:, p, ts(ft, 128)],
                                rhs=OT[:, p, ts(tc_, 512)],
                                start=(p == 0),
                                stop=(p == NP - 1),
                            )
                    for tc_ in range(4):
                        ob = opool.tile([128, 512], F32)
                        nc.scalar.copy(out=ob, in_=pso[tc_])
                        nc.sync.dma_start(out=ot[ts(ft, 128), ts(tc_, 512)], in_=ob)

    nc.compile()
    return nc


def kernel(x, W_qkv, b_qkv, W_out, b_out):
    x = np.asarray(x, dtype=np.float32)
    W_qkv = np.asarray(W_qkv, dtype=np.float32)
    b_qkv = np.asarray(b_qkv, dtype=np.float32)
    W_out = np.asarray(W_out, dtype=np.float32)
    b_out = np.asarray(b_out, dtype=np.float32)
    B = x.shape[0]

    aug = bool(np.any(b_qkv))
    CT = 9 if aug else 8
    key = CT
    if key not in _CACHE:
        _CACHE[key] = _build(CT)
    nc = _CACHE[key]

    # causal keep-mask for diagonal k-tiles: [p, jm, q] = 1 if q >= 128*jm + p
    pidx = np.arange(128)[:, None, None]
    jmidx = np.arange(4)[None, :, None]
    qidx = np.arange(512)[None, None, :]
    dmask = (qidx >= 128 * jmidx + pidx).astype(np.float32)

    in_maps = []
    for core in range(8):
        b, g = core // 2, core % 2
        xa = x[b]
        if aug:
            pad = np.zeros((T, 128), np.float32)
            pad[:, 0] = 1.0
            xa = np.concatenate([xa, pad], axis=1)

        def wslice(col0):
            w = W_qkv[:, col0 + 512 * g : col0 + 512 * g + 512]
            if aug:
                extra = np.zeros((128, 512), np.float32)
                extra[0] = b_qkv[col0 + 512 * g : col0 + 512 * g + 512]
                w = np.concatenate([w, extra], axis=0)
            return np.ascontiguousarray(w)

        in_maps.append(
            {
                "xa": np.ascontiguousarray(xa),
                "wq": wslice(0),
                "wk": wslice(1024),
                "wv": wslice(2048),
                "wo": np.ascontiguousarray(W_out[512 * g : 512 * g + 512, :]),
                "dmask": dmask,
            }
        )

    global _last_in_maps
    _last_in_maps = in_maps
    res = bass_utils.run_bass_kernel_spmd(nc, in_maps, list(range(8))).results
    out = np.empty((B, T, 1024), np.float32)
    for b in range(B):
        acc = res[2 * b]["ot"] + res[2 * b + 1]["ot"]
        out[b] = acc.T + b_out[None, :]
    return out


# revision 8
# speedup vs baseline: 1.1070x; 1.1070x over previous
"""Causal self-attention (B=4, T=2048, D=1024, H=16) on 8 TRN2 NeuronCores.

Sharding: core i = (batch b = i//2, head-group g = i%2). Data parallel on B,
tensor parallel on heads (8 heads per group): qkv_proj columns and out_proj
rows split per head group. Each core computes a partial [D, T] output^T for
its batch; host sums the two group partials per batch, transposes, adds bias.

Per-core pipeline (all matmuls in float32r = FP22, full PE rate at N>=256):
  phase 1: x -> x^T via PE transpose; V = x@Wv (natural [t,d] + ones col);
           Q^T, K^T = (x@Wq)^T via transposed projection, bounced to DRAM.
  phase 2: per head pair p, per q-chunk qc (512), per k-tile j (128):
           S^T[k,q] = K^T.T @ Q^T (heads at partitions 0-63 / 64-127);
           one exp over both heads' strips (trimmed to the causal columns);
           triangle mask-mul on the diagonal 128-block; AV: psum[65,512] +=
           V'[k,d+1].T @ P^T accumulated over j -- row 64 is the softmax
           denominator (ones column). Normalize with reciprocal_approx_fast
           + gpsimd partition_broadcast.
  phase 3: out^T[f,t] = sum_p Wo_pair[d128,f].T @ O^T_pair[d128,t].
"""

import numpy as np

import concourse.bacc as bacc
import concourse.tile as tile
import concourse.mybir as mybir
from concourse import bass_utils
from concourse.bass import ts
from concourse.masks import make_identity

F32 = mybir.dt.float32
F32R = mybir.dt.float32r
EXP = mybir.ActivationFunctionType.Exp

T = 2048
TT = 16          # t tiles of 128
NP = 4           # head pairs per core
NQC = 4          # q chunks of 512
SCALE = 0.125    # 1/sqrt(64)

_CACHE = {}
_last_in_maps = None


def _build(CT):
    """CT = number of 128-row c-tiles in the (possibly bias-augmented) x/W."""
    nc = bacc.Bacc("TRN2", target_bir_lowering=False, debug=False)
    C = CT * 128

    xa = nc.dram_tensor("xa", [T, C], F32, kind="ExternalInput").ap()
    wq = nc.dram_tensor("wq", [C, 512], F32, kind="ExternalInput").ap()
    wk = nc.dram_tensor("wk", [C, 512], F32, kind="ExternalInput").ap()
    wv = nc.dram_tensor("wv", [C, 512], F32, kind="ExternalInput").ap()
    wo = nc.dram_tensor("wo", [512, 1024], F32, kind="ExternalInput").ap()
    tri = nc.dram_tensor("tri", [128, 128], F32, kind="ExternalInput").ap()
    ot = nc.dram_tensor("ot", [1024, T], F32, kind="ExternalOutput").ap()

    with tile.TileContext(nc) as tc:
        with (
            tc.tile_pool(name="persist", bufs=1) as persist,
            tc.tile_pool(name="dram", bufs=1, space="DRAM") as dpool,
        ):
            vS = persist.tile([128, TT, 8, 65], F32R)     # [k128, ktile, head, d+1]
            OT = persist.tile([128, NP, T], F32R)         # [d128(2 heads), pair, t]
            tr = persist.tile([128, 128], F32R)
            wo_sb = persist.tile([128, NP, 1024], F32R)
            nc.vector.memset(vS[:, :, :, 64:65].bitcast(F32), 1.0)

            qTd = dpool.tile([128, NP, T], F32)           # Q^T bounce [f128, pair, t]
            kTd = dpool.tile([128, NP, T], F32)

            # ---------------- phase 1: transpose + projections ----------------
            with (
                tc.tile_pool(name="ph1", bufs=1) as ph1,
                tc.tile_pool(name="xnat", bufs=4) as xnat,
                tc.tile_pool(name="bounce", bufs=4) as bpool,
                tc.tile_pool(name="pst", bufs=2, space="PSUM") as pst,
                tc.tile_pool(name="psp", bufs=4, space="PSUM") as psp,
            ):
                ident = ph1.tile([128, 128], F32)
                make_identity(nc, ident)
                wv_sb = ph1.tile([128, CT, 512], F32R)
                wq_sb = ph1.tile([128, CT, NP, 128], F32R)
                wk_sb = ph1.tile([128, CT, NP, 128], F32R)
                xT = [ph1.tile([128, T], F32R, name=f"xT{cc}") for cc in range(CT)]

                def load_weights():
                    # big strided loads on the gpsimd queue set, emitted after
                    # the first transpose batch so x tiles go out first
                    nc.gpsimd.dma_start(out=tr, in_=tri.bitcast(F32R))
                    nc.gpsimd.dma_start(
                        out=wv_sb,
                        in_=wv.rearrange("(ct P) f -> P ct f", P=128).bitcast(F32R),
                    )
                    nc.gpsimd.dma_start(
                        out=wq_sb,
                        in_=wq.rearrange("(ct P) (np f) -> P ct np f", P=128, np=NP).bitcast(F32R),
                    )
                    nc.gpsimd.dma_start(
                        out=wk_sb,
                        in_=wk.rearrange("(ct P) (np f) -> P ct np f", P=128, np=NP).bitcast(F32R),
                    )
                    nc.gpsimd.dma_start(
                        out=wo_sb,
                        in_=wo.rearrange("(np P) f -> P np f", P=128).bitcast(F32R),
                    )

                def transpose_tt(tt):
                    for cc in range(CT):
                        xn = xnat.tile([128, 128], F32)
                        nc.sync.dma_start(out=xn, in_=xa[ts(tt, 128), ts(cc, 128)])
                        pt_ = pst.tile([128, 512], F32)
                        nc.tensor.transpose(pt_[:, :128], xn, ident)
                        nc.vector.tensor_copy(out=xT[cc][:, ts(tt, 128)], in_=pt_[:, :128])

                def vproj_tt(tt):
                    ps = psp.tile([128, 512], F32)
                    for cc in range(CT):
                        nc.tensor.matmul(
                            ps,
                            lhsT=xT[cc][:, ts(tt, 128)],
                            rhs=wv_sb[:, cc, :],
                            start=(cc == 0),
                            stop=(cc == CT - 1),
                        )
                    nc.vector.tensor_copy(
                        out=vS[:, tt, :, 0:64],
                        in_=ps.rearrange("p (h d) -> p h d", h=8),
                    )

                def qkproj_tc(tc_):
                    for p in range(NP):
                        for w_sb, dst, scl in ((wq_sb, qTd, SCALE), (wk_sb, kTd, 1.0)):
                            ps = psp.tile([128, 512], F32)
                            for cc in range(CT):
                                nc.tensor.matmul(
                                    ps,
                                    lhsT=w_sb[:, cc, p, :],
                                    rhs=xT[cc][:, ts(tc_, 512)],
                                    start=(cc == 0),
                                    stop=(cc == CT - 1),
                                )
                            bo = bpool.tile([128, 512], F32)
                            nc.scalar.mul(out=bo, in_=ps, mul=scl)
                            nc.sync.dma_start(out=dst[:, p, ts(tc_, 512)], in_=bo)

                for tt in range(TT + 1):
                    if tt < TT:
                        transpose_tt(tt)
                    if tt == 0:
                        load_weights()
                    if tt >= 1:
                        vproj_tt(tt - 1)
                        if (tt - 1) % 4 == 3:
                            qkproj_tc((tt - 1) // 4)

            # ---------------- phase 2: attention ----------------
            with (
                tc.tile_pool(name="qk", bufs=2) as qkpool,
                tc.tile_pool(name="ptp", bufs=5) as ptpool,
                tc.tile_pool(name="rsm", bufs=4) as rpool,
                tc.tile_pool(name="rbcp", bufs=4) as rbcpool,
                tc.tile_pool(name="psS", bufs=2, space="PSUM") as psS,
                tc.tile_pool(name="psAv", bufs=4, space="PSUM") as psAv,
            ):
                for p in range(NP):
                    qT = qkpool.tile([128, T], F32R, tag="qT")
                    nc.sync.dma_start(out=qT, in_=qTd[:, p, :].bitcast(F32R))
                    kT = qkpool.tile([128, T], F32R, tag="kT")
                    nc.sync.dma_start(out=kT, in_=kTd[:, p, :].bitcast(F32R))
                    for qc in range(NQC):
                        nj = 4 * qc + 4
                        av = [psAv.tile([128, 512], F32, name="av", tag="av") for _ in range(2)]
                        pts = [None] * nj
                        offs = [0] * nj

                        def s_exp(j):
                            # causal column offset within this q-chunk
                            off = max(0, 128 * j - 512 * qc)
                            sg = psS.tile([128, 2, 512], F32)
                            for m in range(2):
                                nc.tensor.matmul(
                                    sg[:, m, off:],
                                    lhsT=kT[64 * m : 64 * m + 64, ts(j, 128)],
                                    rhs=qT[64 * m : 64 * m + 64, 512 * qc + off : 512 * (qc + 1)],
                                    start=True,
                                    stop=True,
                                )
                            ptile = ptpool.tile([128, 2, 512], F32R)
                            nc.scalar.activation(
                                out=ptile[:, :, off:], in_=sg[:, :, off:], func=EXP
                            )
                            if j >= 4 * qc:
                                # triangle mask on the 128-wide diagonal block
                                nc.vector.tensor_mul(
                                    ptile[:, :, off : off + 128],
                                    ptile[:, :, off : off + 128],
                                    tr[:, None, :].to_broadcast([128, 2, 128]),
                                )
                            pts[j] = ptile
                            offs[j] = off

                        def av_mm(j):
                            off = offs[j]
                            for m in range(2):
                                nc.tensor.matmul(
                                    av[m][:65, off:],
                                    lhsT=vS[:, j, 2 * p + m, :],
                                    rhs=pts[j][:, m, off:],
                                    start=(j == 0),
                                    stop=(j == nj - 1),
                                )
                            pts[j] = None

                        # software pipeline: S/exp two j ahead of AV
                        for j in range(nj + 2):
                            if j < nj:
                                s_exp(j)
                            if j >= 2:
                                av_mm(j - 2)

                        for m in range(2):
                            rsb = rpool.tile([1, 512], F32, name="rsb", tag="rsb")
                            nc.vector.tensor_copy(out=rsb, in_=av[m][64:65, :])
                            rinv = rpool.tile([1, 512], F32)
                            nc.vector.reciprocal_approx_fast(out=rinv, in_=rsb)
                            rb = rbcpool.tile([64, 512], F32)
                            nc.gpsimd.partition_broadcast(rb, rinv)
                            nc.vector.tensor_mul(
                                OT[64 * m : 64 * m + 64, p, ts(qc, 512)],
                                av[m][0:64, :],
                                rb,
                            )

            # ---------------- phase 3: output projection ----------------
            with (
                tc.tile_pool(name="obnc", bufs=4) as opool,
                tc.tile_pool(name="psO", bufs=8, space="PSUM") as psO,
            ):
                for ft in range(8):
                    pso = [psO.tile([128, 512], F32, name="pso", tag="pso") for _ in range(4)]
                    for p in range(NP):
                        for tc_ in range(4):
                            nc.tensor.matmul(
                                pso[tc_],
                                lhsT=wo_sb[:, p, ts(ft, 128)],
                                rhs=OT[:, p, ts(tc_, 512)],
                                start=(p == 0),
                                stop=(p == NP - 1),
                            )
                    for tc_ in range(4):
                        ob = opool.tile([128, 512], F32)
                        nc.vector.tensor_copy(out=ob, in_=pso[tc_])
                        nc.sync.dma_start(out=ot[ts(ft, 128), ts(tc_, 512)], in_=ob)

    nc.compile()
    return nc


def kernel(x, W_qkv, b_qkv, W_out, b_out):
    global _last_in_maps
    x = np.asarray(x, dtype=np.float32)
    W_qkv = np.asarray(W_qkv, dtype=np.float32)
    b_qkv = np.asarray(b_qkv, dtype=np.float32)
    W_out = np.asarray(W_out, dtype=np.float32)
    b_out = np.asarray(b_out, dtype=np.float32)
    B = x.shape[0]

    aug = bool(np.any(b_qkv))
    CT = 9 if aug else 8
    if CT not in _CACHE:
        _CACHE[CT] = _build(CT)
    nc = _CACHE[CT]

    # triangle keep-mask for the diagonal 128 block: [p, c] = 1 if c >= p
    tri = (np.arange(128)[None, :] >= np.arange(128)[:, None]).astype(np.float32)

    in_maps = []
    for core in range(8):
        b, g = core // 2, core % 2
        xa = x[b]
        if aug:
            pad = np.zeros((T, 128), np.float32)
            pad[:, 0] = 1.0
            xa = np.concatenate([xa, pad], axis=1)

        def wslice(col0):
            w = W_qkv[:, col0 + 512 * g : col0 + 512 * g + 512]
            if aug:
                extra = np.zeros((128, 512), np.float32)
                extra[0] = b_qkv[col0 + 512 * g : col0 + 512 * g + 512]
                w = np.concatenate([w, extra], axis=0)
            return np.ascontiguousarray(w)

        in_maps.append(
            {
                "xa": np.ascontiguousarray(xa),
                "wq": wslice(0),
                "wk": wslice(1024),
                "wv": wslice(2048),
                "wo": np.ascontiguousarray(W_out[512 * g : 512 * g + 512, :]),
                "tri": tri,
            }
        )

    _last_in_maps = in_maps
    res = bass_utils.run_bass_kernel_spmd(nc, in_maps, list(range(8))).results
    out = np.empty((B, T, 1024), np.float32)
    for b in range(B):
        acc = res[2 * b]["ot"] + res[2 * b + 1]["ot"]
        out[b] = acc.T + b_out[None, :]
    return out


# revision 9
# speedup vs baseline: 1.1593x; 1.0473x over previous
"""Causal self-attention (B=4, T=2048, D=1024, H=16) on 8 TRN2 NeuronCores.

Sharding: core i = (batch b = i//2, head-group g = i%2). Data parallel on B,
tensor parallel on heads (8 heads per group): qkv_proj columns and out_proj
rows split per head group. Each core computes a partial [D, T] output^T for
its batch; host sums the two group partials per batch, transposes, adds bias.

Per-core pipeline (all matmuls in float32r = FP22, full PE rate at N>=256):
  phase 1: x -> x^T via PE transpose; V = x@Wv (natural [t,d] + ones col);
           Q^T, K^T = (x@Wq)^T via transposed projection, bounced to DRAM.
  phase 2: per head pair p, per q-chunk qc (512), per k-tile j (128):
           S^T[k,q] = K^T.T @ Q^T (heads at partitions 0-63 / 64-127);
           one exp over both heads' strips (trimmed to the causal columns);
           triangle mask-mul on the diagonal 128-block; AV: psum[65,512] +=
           V'[k,d+1].T @ P^T accumulated over j -- row 64 is the softmax
           denominator (ones column). Normalize with reciprocal_approx_fast
           + gpsimd partition_broadcast.
  phase 3: out^T[f,t] = sum_p Wo_pair[d128,f].T @ O^T_pair[d128,t].
"""

import numpy as np

import concourse.bacc as bacc
import concourse.tile as tile
import concourse.mybir as mybir
from concourse import bass_utils
from concourse.bass import ts
from concourse.masks import make_identity

F32 = mybir.dt.float32
F32R = mybir.dt.float32r
EXP = mybir.ActivationFunctionType.Exp

T = 2048
TT = 16          # t tiles of 128
NP = 4           # head pairs per core
NQC = 4          # q chunks of 512
SCALE = 0.125    # 1/sqrt(64)

_CACHE = {}
_last_in_maps = None


def _build(CT):
    """CT = number of 128-row c-tiles in the (possibly bias-augmented) x/W."""
    nc = bacc.Bacc("TRN2", target_bir_lowering=False, debug=False)
    C = CT * 128

    xa = nc.dram_tensor("xa", [T, C], F32, kind="ExternalInput").ap()
    wq = nc.dram_tensor("wq", [C, 512], F32, kind="ExternalInput").ap()
    wk = nc.dram_tensor("wk", [C, 512], F32, kind="ExternalInput").ap()
    wv = nc.dram_tensor("wv", [C, 512], F32, kind="ExternalInput").ap()
    wo = nc.dram_tensor("wo", [512, 1024], F32, kind="ExternalInput").ap()
    tri = nc.dram_tensor("tri", [128, 128], F32, kind="ExternalInput").ap()
    ot = nc.dram_tensor("ot", [1024, T], F32, kind="ExternalOutput").ap()

    with tile.TileContext(nc) as tc:
        with (
            tc.tile_pool(name="persist", bufs=1) as persist,
            tc.tile_pool(name="dram", bufs=1, space="DRAM") as dpool,
        ):
            vS = persist.tile([128, TT, 8, 65], F32R)     # [k128, ktile, head, d+1]
            OT = persist.tile([128, NP, T], F32R)         # [d128(2 heads), pair, t]
            tr = persist.tile([128, 128], F32R)
            wo_sb = persist.tile([128, NP, 1024], F32R)
            nc.vector.memset(vS[:, :, :, 64:65].bitcast(F32), 1.0)

            # Q^T / K^T bounce chunks, one DRAM tile per (pair, t-chunk) so a
            # chunk becomes readable as soon as its projection lands
            qtd = {}
            ktd = {}
            for _p in range(NP):
                for _tc in range(4):
                    qtd[(_p, _tc)] = dpool.tile([128, 512], F32, name=f"qtd{_p}{_tc}")
                    ktd[(_p, _tc)] = dpool.tile([128, 512], F32, name=f"ktd{_p}{_tc}")

            # ---------------- phase 1: transpose + projections ----------------
            with (
                tc.tile_pool(name="ph1", bufs=1) as ph1,
                tc.tile_pool(name="xnat", bufs=4) as xnat,
                tc.tile_pool(name="bounce", bufs=4) as bpool,
                tc.tile_pool(name="pst", bufs=2, space="PSUM") as pst,
                tc.tile_pool(name="psp", bufs=4, space="PSUM") as psp,
            ):
                ident = ph1.tile([128, 128], F32)
                make_identity(nc, ident)
                wv_sb = ph1.tile([128, CT, 512], F32R)
                wq_sb = ph1.tile([128, CT, NP, 128], F32R)
                wk_sb = ph1.tile([128, CT, NP, 128], F32R)
                xT = [ph1.tile([128, T], F32R, name=f"xT{cc}") for cc in range(CT)]

                def load_weights():
                    # big strided loads on the gpsimd queue set, emitted after
                    # the first transpose batch so x tiles go out first
                    nc.gpsimd.dma_start(out=tr, in_=tri.bitcast(F32R))
                    nc.gpsimd.dma_start(
                        out=wv_sb,
                        in_=wv.rearrange("(ct P) f -> P ct f", P=128).bitcast(F32R),
                    )
                    nc.gpsimd.dma_start(
                        out=wq_sb,
                        in_=wq.rearrange("(ct P) (np f) -> P ct np f", P=128, np=NP).bitcast(F32R),
                    )
                    nc.gpsimd.dma_start(
                        out=wk_sb,
                        in_=wk.rearrange("(ct P) (np f) -> P ct np f", P=128, np=NP).bitcast(F32R),
                    )
                    nc.gpsimd.dma_start(
                        out=wo_sb,
                        in_=wo.rearrange("(np P) f -> P np f", P=128).bitcast(F32R),
                    )

                def transpose_tt(tt):
                    for cc in range(CT):
                        xn = xnat.tile([128, 128], F32)
                        nc.sync.dma_start(out=xn, in_=xa[ts(tt, 128), ts(cc, 128)])
                        pt_ = pst.tile([128, 512], F32)
                        nc.tensor.transpose(pt_[:, :128], xn, ident)
                        nc.vector.tensor_copy(out=xT[cc][:, ts(tt, 128)], in_=pt_[:, :128])

                def vproj_tt(tt):
                    ps = psp.tile([128, 512], F32)
                    for cc in range(CT):
                        nc.tensor.matmul(
                            ps,
                            lhsT=xT[cc][:, ts(tt, 128)],
                            rhs=wv_sb[:, cc, :],
                            start=(cc == 0),
                            stop=(cc == CT - 1),
                        )
                    nc.vector.tensor_copy(
                        out=vS[:, tt, :, 0:64],
                        in_=ps.rearrange("p (h d) -> p h d", h=8),
                    )

                def qkproj_tc(tc_):
                    for p in range(NP):
                        for w_sb, dst, scl in ((wq_sb, qtd, SCALE), (wk_sb, ktd, 1.0)):
                            ps = psp.tile([128, 512], F32)
                            for cc in range(CT):
                                nc.tensor.matmul(
                                    ps,
                                    lhsT=w_sb[:, cc, p, :],
                                    rhs=xT[cc][:, ts(tc_, 512)],
                                    start=(cc == 0),
                                    stop=(cc == CT - 1),
                                )
                            bo = bpool.tile([128, 512], F32)
                            nc.scalar.mul(out=bo, in_=ps, mul=scl)
                            nc.sync.dma_start(out=dst[(p, tc_)], in_=bo)

                for tt in range(TT + 1):
                    if tt < TT:
                        transpose_tt(tt)
                    if tt == 0:
                        load_weights()
                    if tt >= 1:
                        vproj_tt(tt - 1)
                        if (tt - 1) % 4 == 3:
                            qkproj_tc((tt - 1) // 4)

            # ---------------- phase 2: attention ----------------
            # Emission keeps the PE in same-type runs: a group of 3 j-steps of
            # S matmuls (+exp on ACT), then the previous group's AV matmuls.
            # Interleaving S/AV per-j costs ~25%/MM in PE streaming rate.
            with (
                tc.tile_pool(name="qkc", bufs=8) as qkcpool,
                tc.tile_pool(name="ptp", bufs=10) as ptpool,
                tc.tile_pool(name="rsm", bufs=4) as rpool,
                tc.tile_pool(name="rbcp", bufs=4) as rbcpool,
                tc.tile_pool(name="psS", bufs=3, space="PSUM") as psS,
                tc.tile_pool(name="psAv", bufs=2, space="PSUM") as psAv,
            ):
                qch = {}
                kch = {}
                avs = {}
                pts = {}

                def fetch_pair(p):
                    for tc_ in range(4):
                        qt = qkcpool.tile([128, 512], F32R, name="qTc", tag="qTc")
                        nc.sync.dma_start(out=qt, in_=qtd[(p, tc_)].bitcast(F32R))
                        qch[(p, tc_)] = qt
                        kt = qkcpool.tile([128, 512], F32R, name="kTc", tag="kTc")
                        nc.sync.dma_start(out=kt, in_=ktd[(p, tc_)].bitcast(F32R))
                        kch[(p, tc_)] = kt

                def s_exp(p, qc, j):
                    off = max(0, 128 * j - 512 * qc)
                    sg = psS.tile([128, 2, 512], F32)
                    kc = kch[(p, j // 4)]
                    qc_t = qch[(p, qc)]
                    jo = 128 * (j % 4)
                    for m in range(2):
                        nc.tensor.matmul(
                            sg[:, m, off:],
                            lhsT=kc[64 * m : 64 * m + 64, jo : jo + 128],
                            rhs=qc_t[64 * m : 64 * m + 64, off:],
                            start=True,
                            stop=True,
                        )
                    ptile = ptpool.tile([128, 2, 512], F32R)
                    nc.scalar.activation(
                        out=ptile[:, :, off:], in_=sg[:, :, off:], func=EXP
                    )
                    if j >= 4 * qc:
                        nc.vector.tensor_mul(
                            ptile[:, :, off : off + 128],
                            ptile[:, :, off : off + 128],
                            tr[:, None, :].to_broadcast([128, 2, 128]),
                        )
                    pts[(p, qc, j)] = (ptile, off)

                def av_mm(p, qc, j, nj):
                    ptile, off = pts.pop((p, qc, j))
                    av = avs[(p, qc)]
                    for m in range(2):
                        nc.tensor.matmul(
                            av[m][:65, off:],
                            lhsT=vS[:, j, 2 * p + m, :],
                            rhs=ptile[:, m, off:],
                            start=(j == 0),
                            stop=(j == nj - 1),
                        )

                def normalize(p, qc):
                    av = avs.pop((p, qc))
                    for m in range(2):
                        rsb = rpool.tile([1, 512], F32, name="rsb", tag="rsb")
                        nc.vector.tensor_copy(out=rsb, in_=av[m][64:65, :])
                        rinv = rpool.tile([1, 512], F32, name="rinv", tag="rinv")
                        nc.vector.reciprocal_approx_fast(out=rinv, in_=rsb)
                        rb = rbcpool.tile([64, 512], F32, name="rb", tag="rb")
                        nc.gpsimd.partition_broadcast(rb, rinv)
                        nc.vector.tensor_mul(
                            OT[64 * m : 64 * m + 64, p, ts(qc, 512)],
                            av[m][0:64, :],
                            rb,
                        )

                groups = []
                for p in range(NP):
                    for qc in range(NQC):
                        nj = 4 * qc + 4
                        js = list(range(nj))
                        sub = [js[i : i + 3] for i in range(0, nj, 3)]
                        for gi, jg in enumerate(sub):
                            groups.append((p, qc, nj, jg, gi == 0, gi == len(sub) - 1))

                for i in range(len(groups) + 1):
                    if i < len(groups):
                        p, qc, nj, jg, first, last = groups[i]
                        if qc == 0 and first:
                            fetch_pair(p)
                        for j in jg:
                            s_exp(p, qc, j)
                    if i >= 1:
                        p, qc, nj, jg, first, last = groups[i - 1]
                        if first:
                            avs[(p, qc)] = [
                                psAv.tile([128, 512], F32, name="av", tag="av")
                                for _ in range(2)
                            ]
                        for j in jg:
                            av_mm(p, qc, j, nj)
                        if last:
                            normalize(p, qc)

            # ---------------- phase 3: output projection ----------------
            with (
                tc.tile_pool(name="obnc", bufs=4) as opool,
                tc.tile_pool(name="psO", bufs=8, space="PSUM") as psO,
            ):
                for ft in range(8):
                    pso = [psO.tile([128, 512], F32, name="pso", tag="pso") for _ in range(4)]
                    for p in range(NP):
                        for tc_ in range(4):
                            nc.tensor.matmul(
                                pso[tc_],
                                lhsT=wo_sb[:, p, ts(ft, 128)],
                                rhs=OT[:, p, ts(tc_, 512)],
                                start=(p == 0),
                                stop=(p == NP - 1),
                            )
                    for tc_ in range(4):
                        ob = opool.tile([128, 512], F32)
                        nc.vector.tensor_copy(out=ob, in_=pso[tc_])
                        nc.sync.dma_start(out=ot[ts(ft, 128), ts(tc_, 512)], in_=ob)

    nc.compile()
    return nc


def kernel(x, W_qkv, b_qkv, W_out, b_out):
    global _last_in_maps
    x = np.asarray(x, dtype=np.float32)
    W_qkv = np.asarray(W_qkv, dtype=np.float32)
    b_qkv = np.asarray(b_qkv, dtype=np.float32)
    W_out = np.asarray(W_out, dtype=np.float32)
    b_out = np.asarray(b_out, dtype=np.float32)
    B = x.shape[0]

    aug = bool(np.any(b_qkv))
    CT = 9 if aug else 8
    if CT not in _CACHE:
        _CACHE[CT] = _build(CT)
    nc = _CACHE[CT]

    # triangle keep-mask for the diagonal 128 block: [p, c] = 1 if c >= p
    tri = (np.arange(128)[None, :] >= np.arange(128)[:, None]).astype(np.float32)

    in_maps = []
    for core in range(8):
        b, g = core // 2, core % 2
        xa = x[b]
        if aug:
            pad = np.zeros((T, 128), np.float32)
            pad[:, 0] = 1.0
            xa = np.concatenate([xa, pad], axis=1)

        def wslice(col0):
            w = W_qkv[:, col0 + 512 * g : col0 + 512 * g + 512]
            if aug:
                extra = np.zeros((128, 512), np.float32)
                extra[0] = b_qkv[col0 + 512 * g : col0 + 512 * g + 512]
                w = np.concatenate([w, extra], axis=0)
            return np.ascontiguousarray(w)

        in_maps.append(
            {
                "xa": np.ascontiguousarray(xa),
                "wq": wslice(0),
                "wk": wslice(1024),
                "wv": wslice(2048),
                "wo": np.ascontiguousarray(W_out[512 * g : 512 * g + 512, :]),
                "tri": tri,
            }
        )

    _last_in_maps = in_maps
    res = bass_utils.run_bass_kernel_spmd(nc, in_maps, list(range(8))).results
    out = np.empty((B, T, 1024), np.float32)
    for b in range(B):
        acc = res[2 * b]["ot"] + res[2 * b + 1]["ot"]
        out[b] = acc.T + b_out[None, :]
    return out


# revision 10
# speedup vs baseline: 1.1649x; 1.0048x over previous
"""Causal self-attention (B=4, T=2048, D=1024, H=16) on 8 TRN2 NeuronCores.

Sharding: core i = (batch b = i//2, head-group g = i%2). Data parallel on B,
tensor parallel on heads (8 heads per group): qkv_proj columns and out_proj
rows split per head group. Each core computes a partial [D, T] output^T for
its batch; host sums the two group partials per batch, transposes, adds bias.

Per-core pipeline (all matmuls in float32r = FP22, full PE rate at N>=256):
  phase 1: x -> x^T via PE transpose; V = x@Wv (natural [t,d] + ones col);
           Q^T, K^T = (x@Wq)^T via transposed projection, bounced to DRAM.
  phase 2: per head pair p, per q-chunk qc (512), per k-tile j (128):
           S^T[k,q] = K^T.T @ Q^T (heads at partitions 0-63 / 64-127);
           one exp over both heads' strips (trimmed to the causal columns);
           triangle mask-mul on the diagonal 128-block; AV: psum[65,512] +=
           V'[k,d+1].T @ P^T accumulated over j -- row 64 is the softmax
           denominator (ones column). Normalize with reciprocal_approx_fast
           + gpsimd partition_broadcast.
  phase 3: out^T[f,t] = sum_p Wo_pair[d128,f].T @ O^T_pair[d128,t].
"""

import numpy as np

import concourse.bacc as bacc
import concourse.tile as tile
import concourse.mybir as mybir
from concourse import bass_utils
from concourse.bass import ts
from concourse.masks import make_identity

F32 = mybir.dt.float32
F32R = mybir.dt.float32r
EXP = mybir.ActivationFunctionType.Exp

T = 2048
TT = 16          # t tiles of 128
NP = 4           # head pairs per core
NQC = 4          # q chunks of 512
SCALE = 0.125    # 1/sqrt(64)

_CACHE = {}
_last_in_maps = None


def _build(CT):
    """CT = number of 128-row c-tiles in the (possibly bias-augmented) x/W."""
    nc = bacc.Bacc("TRN2", target_bir_lowering=False, debug=False)
    C = CT * 128

    xa = nc.dram_tensor("xa", [T, C], F32, kind="ExternalInput").ap()
    wq = nc.dram_tensor("wq", [C, 512], F32, kind="ExternalInput").ap()
    wk = nc.dram_tensor("wk", [C, 512], F32, kind="ExternalInput").ap()
    wv = nc.dram_tensor("wv", [C, 512], F32, kind="ExternalInput").ap()
    wo = nc.dram_tensor("wo", [512, 1024], F32, kind="ExternalInput").ap()
    tri = nc.dram_tensor("tri", [128, 128], F32, kind="ExternalInput").ap()
    ot = nc.dram_tensor("ot", [1024, T], F32, kind="ExternalOutput").ap()

    with tile.TileContext(nc) as tc:
        with (
            tc.tile_pool(name="persist", bufs=1) as persist,
            tc.tile_pool(name="dram", bufs=1, space="DRAM") as dpool,
        ):
            vS = persist.tile([128, TT, 8, 65], F32R)     # [k128, ktile, head, d+1]
            OT = persist.tile([128, NP, T], F32R)         # [d128(2 heads), pair, t]
            tr = persist.tile([128, 128], F32R)
            wo_sb = persist.tile([128, NP, 1024], F32R)
            nc.vector.memset(vS[:, :, :, 64:65].bitcast(F32), 1.0)

            # Q^T / K^T bounce chunks, one DRAM tile per (pair, t-chunk) so a
            # chunk becomes readable as soon as its projection lands
            qtd = {}
            ktd = {}
            for _p in range(NP):
                for _tc in range(4):
                    qtd[(_p, _tc)] = dpool.tile([128, 512], F32, name=f"qtd{_p}{_tc}")
                    ktd[(_p, _tc)] = dpool.tile([128, 512], F32, name=f"ktd{_p}{_tc}")

            # ---------------- phase 1: transpose + projections ----------------
            with (
                tc.tile_pool(name="ph1", bufs=1) as ph1,
                tc.tile_pool(name="xnat", bufs=4) as xnat,
                tc.tile_pool(name="bounce", bufs=4) as bpool,
                tc.tile_pool(name="pst", bufs=2, space="PSUM") as pst,
                tc.tile_pool(name="psp", bufs=4, space="PSUM") as psp,
            ):
                ident = ph1.tile([128, 128], F32)
                make_identity(nc, ident)
                wv_sb = ph1.tile([128, CT, 512], F32R)
                wq_sb = ph1.tile([128, CT, NP, 128], F32R)
                wk_sb = ph1.tile([128, CT, NP, 128], F32R)
                xT = [ph1.tile([128, T], F32R, name=f"xT{cc}") for cc in range(CT)]

                def load_weights():
                    # big strided loads on the gpsimd queue set, emitted after
                    # the first transpose batch so x tiles go out first
                    nc.gpsimd.dma_start(out=tr, in_=tri.bitcast(F32R))
                    nc.gpsimd.dma_start(
                        out=wv_sb,
                        in_=wv.rearrange("(ct P) f -> P ct f", P=128).bitcast(F32R),
                    )
                    nc.gpsimd.dma_start(
                        out=wq_sb,
                        in_=wq.rearrange("(ct P) (np f) -> P ct np f", P=128, np=NP).bitcast(F32R),
                    )
                    nc.gpsimd.dma_start(
                        out=wk_sb,
                        in_=wk.rearrange("(ct P) (np f) -> P ct np f", P=128, np=NP).bitcast(F32R),
                    )
                    nc.gpsimd.dma_start(
                        out=wo_sb,
                        in_=wo.rearrange("(np P) f -> P np f", P=128).bitcast(F32R),
                    )

                def transpose_tt(tt):
                    for cc in range(CT):
                        xn = xnat.tile([128, 128], F32)
                        nc.sync.dma_start(out=xn, in_=xa[ts(tt, 128), ts(cc, 128)])
                        pt_ = pst.tile([128, 512], F32)
                        nc.tensor.transpose(pt_[:, :128], xn, ident)
                        nc.vector.tensor_copy(out=xT[cc][:, ts(tt, 128)], in_=pt_[:, :128])

                def vproj_tt(tt):
                    ps = psp.tile([128, 512], F32)
                    for cc in range(CT):
                        nc.tensor.matmul(
                            ps,
                            lhsT=xT[cc][:, ts(tt, 128)],
                            rhs=wv_sb[:, cc, :],
                            start=(cc == 0),
                            stop=(cc == CT - 1),
                        )
                    nc.vector.tensor_copy(
                        out=vS[:, tt, :, 0:64],
                        in_=ps.rearrange("p (h d) -> p h d", h=8),
                    )

                def qkproj_tc(tc_):
                    for p in range(NP):
                        for w_sb, dst, scl in ((wq_sb, qtd, SCALE), (wk_sb, ktd, 1.0)):
                            ps = psp.tile([128, 512], F32)
                            for cc in range(CT):
                                nc.tensor.matmul(
                                    ps,
                                    lhsT=w_sb[:, cc, p, :],
                                    rhs=xT[cc][:, ts(tc_, 512)],
                                    start=(cc == 0),
                                    stop=(cc == CT - 1),
                                )
                            bo = bpool.tile([128, 512], F32)
                            nc.scalar.mul(out=bo, in_=ps, mul=scl)
                            nc.sync.dma_start(out=dst[(p, tc_)], in_=bo)

                for tt in range(TT + 1):
                    if tt < TT:
                        transpose_tt(tt)
                    if tt == 0:
                        load_weights()
                    if tt >= 1:
                        vproj_tt(tt - 1)
                        if (tt - 1) % 4 == 3:
                            qkproj_tc((tt - 1) // 4)

            # ---------------- phase 2: attention ----------------
            # Emission keeps the PE in same-type runs: a group of 3 j-steps of
            # S matmuls (+exp on ACT), then the previous group's AV matmuls.
            # Interleaving S/AV per-j costs ~25%/MM in PE streaming rate.
            with (
                tc.tile_pool(name="qkc", bufs=8) as qkcpool,
                tc.tile_pool(name="ptp", bufs=10) as ptpool,
                tc.tile_pool(name="rsm", bufs=4) as rpool,
                tc.tile_pool(name="rbcp", bufs=4) as rbcpool,
                tc.tile_pool(name="psS", bufs=3, space="PSUM") as psS,
                tc.tile_pool(name="psAv", bufs=2, space="PSUM") as psAv,
            ):
                qch = {}
                kch = {}
                avs = {}
                pts = {}

                def fetch_pair(p):
                    for tc_ in range(4):
                        qt = qkcpool.tile([128, 512], F32R, name="qTc", tag="qTc")
                        nc.sync.dma_start(out=qt, in_=qtd[(p, tc_)].bitcast(F32R))
                        qch[(p, tc_)] = qt
                        kt = qkcpool.tile([128, 512], F32R, name="kTc", tag="kTc")
                        nc.sync.dma_start(out=kt, in_=ktd[(p, tc_)].bitcast(F32R))
                        kch[(p, tc_)] = kt

                def s_exp(p, qc, j):
                    off = max(0, 128 * j - 512 * qc)
                    sg = psS.tile([128, 2, 512], F32)
                    kc = kch[(p, j // 4)]
                    qc_t = qch[(p, qc)]
                    jo = 128 * (j % 4)
                    for m in range(2):
                        nc.tensor.matmul(
                            sg[:, m, off:],
                            lhsT=kc[64 * m : 64 * m + 64, jo : jo + 128],
                            rhs=qc_t[64 * m : 64 * m + 64, off:],
                            start=True,
                            stop=True,
                        )
                    ptile = ptpool.tile([128, 2, 512], F32R)
                    nc.scalar.activation(
                        out=ptile[:, :, off:], in_=sg[:, :, off:], func=EXP
                    )
                    if j >= 4 * qc:
                        nc.vector.tensor_mul(
                            ptile[:, :, off : off + 128],
                            ptile[:, :, off : off + 128],
                            tr[:, None, :].to_broadcast([128, 2, 128]),
                        )
                    pts[(p, qc, j)] = (ptile, off)

                def av_mm(p, qc, j, nj):
                    ptile, off = pts.pop((p, qc, j))
                    av = avs[(p, qc)]
                    for m in range(2):
                        nc.tensor.matmul(
                            av[m][:65, off:],
                            lhsT=vS[:, j, 2 * p + m, :],
                            rhs=ptile[:, m, off:],
                            start=(j == 0),
                            stop=(j == nj - 1),
                        )

                def normalize(p, qc):
                    av = avs.pop((p, qc))
                    for m in range(2):
                        rsb = rpool.tile([1, 512], F32, name="rsb", tag="rsb")
                        nc.vector.tensor_copy(out=rsb, in_=av[m][64:65, :])
                        rinv = rpool.tile([1, 512], F32, name="rinv", tag="rinv")
                        nc.vector.reciprocal_approx_fast(out=rinv, in_=rsb)
                        rb = rbcpool.tile([64, 512], F32, name="rb", tag="rb")
                        nc.gpsimd.partition_broadcast(rb, rinv)
                        nc.vector.tensor_mul(
                            OT[64 * m : 64 * m + 64, p, ts(qc, 512)],
                            av[m][0:64, :],
                            rb,
                        )

                groups = []
                for p in range(NP):
                    for qc in range(NQC):
                        nj = 4 * qc + 4
                        js = list(range(nj))
                        sub = [js[i : i + 3] for i in range(0, nj, 3)]
                        for gi, jg in enumerate(sub):
                            groups.append((p, qc, nj, jg, gi == 0, gi == len(sub) - 1))

                LAG = 2
                for i in range(len(groups) + LAG):
                    if i < len(groups):
                        p, qc, nj, jg, first, last = groups[i]
                        if qc == 0 and first:
                            fetch_pair(p)
                        for j in jg:
                            s_exp(p, qc, j)
                    if i >= LAG:
                        p, qc, nj, jg, first, last = groups[i - LAG]
                        if first:
                            avs[(p, qc)] = [
                                psAv.tile([128, 512], F32, name="av", tag="av")
                                for _ in range(2)
                            ]
                        for j in jg:
                            av_mm(p, qc, j, nj)
                        if last:
                            normalize(p, qc)

            # ---------------- phase 3: output projection ----------------
            with (
                tc.tile_pool(name="obnc", bufs=4) as opool,
                tc.tile_pool(name="psO", bufs=8, space="PSUM") as psO,
            ):
                for ft in range(8):
                    pso = [psO.tile([128, 512], F32, name="pso", tag="pso") for _ in range(4)]
                    for p in range(NP):
                        for tc_ in range(4):
                            nc.tensor.matmul(
                                pso[tc_],
                                lhsT=wo_sb[:, p, ts(ft, 128)],
                                rhs=OT[:, p, ts(tc_, 512)],
                                start=(p == 0),
                                stop=(p == NP - 1),
                            )
                    for tc_ in range(4):
                        ob = opool.tile([128, 512], F32)
                        nc.vector.tensor_copy(out=ob, in_=pso[tc_])
                        nc.sync.dma_start(out=ot[ts(ft, 128), ts(tc_, 512)], in_=ob)

    nc.compile()
    return nc


def kernel(x, W_qkv, b_qkv, W_out, b_out):
    global _last_in_maps
    x = np.asarray(x, dtype=np.float32)
    W_qkv = np.asarray(W_qkv, dtype=np.float32)
    b_qkv = np.asarray(b_qkv, dtype=np.float32)
    W_out = np.asarray(W_out, dtype=np.float32)
    b_out = np.asarray(b_out, dtype=np.float32)
    B = x.shape[0]

    aug = bool(np.any(b_qkv))
    CT = 9 if aug else 8
    if CT not in _CACHE:
        _CACHE[CT] = _build(CT)
    nc = _CACHE[CT]

    # triangle keep-mask for the diagonal 128 block: [p, c] = 1 if c >= p
    tri = (np.arange(128)[None, :] >= np.arange(128)[:, None]).astype(np.float32)

    in_maps = []
    for core in range(8):
        b, g = core // 2, core % 2
        xa = x[b]
        if aug:
            pad = np.zeros((T, 128), np.float32)
            pad[:, 0] = 1.0
            xa = np.concatenate([xa, pad], axis=1)

        def wslice(col0):
            w = W_qkv[:, col0 + 512 * g : col0 + 512 * g + 512]
            if aug:
                extra = np.zeros((128, 512), np.float32)
                extra[0] = b_qkv[col0 + 512 * g : col0 + 512 * g + 512]
                w = np.concatenate([w, extra], axis=0)
            return np.ascontiguousarray(w)

        in_maps.append(
            {
                "xa": np.ascontiguousarray(xa),
                "wq": wslice(0),
                "wk": wslice(1024),
                "wv": wslice(2048),
                "wo": np.ascontiguousarray(W_out[512 * g : 512 * g + 512, :]),
                "tri": tri,
            }
        )

    _last_in_maps = in_maps
    res = bass_utils.run_bass_kernel_spmd(nc, in_maps, list(range(8))).results
    out = np.empty((B, T, 1024), np.float32)
    for b in range(B):
        acc = res[2 * b]["ot"] + res[2 * b + 1]["ot"]
        out[b] = acc.T + b_out[None, :]
    return out


# revision 15
# speedup vs baseline: 1.1934x; 1.0245x over previous
"""Causal self-attention (B=4, T=2048, D=1024, H=16) on 8 TRN2 NeuronCores.

Sharding: core i = (batch b = i//2, head-group g = i%2). Data parallel on B,
tensor parallel on heads (8 heads per group): qkv_proj columns and out_proj
rows split per head group. Each core computes a partial [D, T] output^T for
its batch; host sums the two group partials per batch, transposes, adds bias.

Per-core pipeline (all matmuls in float32r = FP22, full PE rate at N>=256):
  phase 1: x -> x^T via PE transpose; V = x@Wv (natural [t,d] + ones col);
           Q^T, K^T = (x@Wq)^T via transposed projection, bounced to DRAM.
  phase 2: per head pair p, per q-chunk qc (512), per k-tile j (128):
           S^T[k,q] = K^T.T @ Q^T (heads at partitions 0-63 / 64-127);
           one exp over both heads' strips (trimmed to the causal columns);
           triangle mask-mul on the diagonal 128-block; AV: psum[65,512] +=
           V'[k,d+1].T @ P^T accumulated over j -- row 64 is the softmax
           denominator (ones column). Normalize with reciprocal_approx_fast
           + gpsimd partition_broadcast.
  phase 3: out^T[f,t] = sum_p Wo_pair[d128,f].T @ O^T_pair[d128,t].
"""

import numpy as np

import concourse.bacc as bacc
import concourse.tile as tile
import concourse.mybir as mybir
from concourse import bass_utils
from concourse.bass import ts
from concourse.masks import make_identity

F32 = mybir.dt.float32
F32R = mybir.dt.float32r
EXP = mybir.ActivationFunctionType.Exp

T = 2048
TT = 16          # t tiles of 128
NP = 4           # head pairs per core
NQC = 4          # q chunks of 512
SCALE = 0.125    # 1/sqrt(64)

_CACHE = {}
_last_in_maps = None


def _build(CT):
    """CT = number of 128-row c-tiles in the (possibly bias-augmented) x/W."""
    nc = bacc.Bacc("TRN2", target_bir_lowering=False, debug=False)
    C = CT * 128

    xa = nc.dram_tensor("xa", [T, C], F32, kind="ExternalInput").ap()
    wq = nc.dram_tensor("wq", [C, 512], F32, kind="ExternalInput").ap()
    wk = nc.dram_tensor("wk", [C, 512], F32, kind="ExternalInput").ap()
    wv = nc.dram_tensor("wv", [C, 512], F32, kind="ExternalInput").ap()
    wo = nc.dram_tensor("wo", [512, 1024], F32, kind="ExternalInput").ap()
    tri = nc.dram_tensor("tri", [128, 128], F32, kind="ExternalInput").ap()
    ot = nc.dram_tensor("ot", [1024, T], F32, kind="ExternalOutput").ap()

    with tile.TileContext(nc) as tc:
        with (
            tc.tile_pool(name="persist", bufs=1) as persist,
            tc.tile_pool(name="dram", bufs=1, space="DRAM") as dpool,
        ):
            vS = persist.tile([128, TT, 8, 65], F32R)     # [k128, ktile, head, d+1]
            OT = persist.tile([128, NP, T], F32R)         # [d128(2 heads), pair, t]
            tr = persist.tile([128, 128], F32R)
            wo_sb = persist.tile([128, NP, 1024], F32R)
            nc.vector.memset(vS[:, :, :, 64:65].bitcast(F32), 1.0)

            # Q^T / K^T bounce chunks, one DRAM tile per (pair, t-chunk) so a
            # chunk becomes readable as soon as its projection lands
            qtd = {}
            ktd = {}
            for _p in range(NP):
                for _tc in range(4):
                    qtd[(_p, _tc)] = dpool.tile([128, 512], F32, name=f"qtd{_p}{_tc}")
                    ktd[(_p, _tc)] = dpool.tile([128, 512], F32, name=f"ktd{_p}{_tc}")

            # ---------------- phase 1: transpose + projections ----------------
            with (
                tc.tile_pool(name="ph1", bufs=1) as ph1,
                tc.tile_pool(name="xnat", bufs=4) as xnat,
                tc.tile_pool(name="bounce", bufs=4) as bpool,
                tc.tile_pool(name="pst", bufs=2, space="PSUM") as pst,
                tc.tile_pool(name="psp", bufs=4, space="PSUM") as psp,
            ):
                ident = ph1.tile([128, 128], F32)
                make_identity(nc, ident)
                wv_sb = ph1.tile([128, CT, 512], F32R)
                wq_sb = ph1.tile([128, CT, NP, 128], F32R)
                wk_sb = ph1.tile([128, CT, NP, 128], F32R)
                xT = [ph1.tile([128, T], F32R, name=f"xT{cc}") for cc in range(CT)]

                def load_weights():
                    # big strided loads on the gpsimd queue set, emitted after
                    # the first transpose batch so x tiles go out first
                    nc.gpsimd.dma_start(out=tr, in_=tri.bitcast(F32R))
                    nc.gpsimd.dma_start(
                        out=wv_sb,
                        in_=wv.rearrange("(ct P) f -> P ct f", P=128).bitcast(F32R),
                    )
                    nc.gpsimd.dma_start(
                        out=wq_sb,
                        in_=wq.rearrange("(ct P) (np f) -> P ct np f", P=128, np=NP).bitcast(F32R),
                    )
                    nc.gpsimd.dma_start(
                        out=wk_sb,
                        in_=wk.rearrange("(ct P) (np f) -> P ct np f", P=128, np=NP).bitcast(F32R),
                    )
                    nc.gpsimd.dma_start(
                        out=wo_sb,
                        in_=wo.rearrange("(np P) f -> P np f", P=128).bitcast(F32R),
                    )

                def transpose_tt(tt):
                    for cc in range(CT):
                        xn = xnat.tile([128, 128], F32)
                        nc.sync.dma_start(out=xn, in_=xa[ts(tt, 128), ts(cc, 128)])
                        pt_ = pst.tile([128, 512], F32)
                        nc.tensor.transpose(pt_[:, :128], xn, ident)
                        nc.vector.tensor_copy(out=xT[cc][:, ts(tt, 128)], in_=pt_[:, :128])

                def vproj_tt(tt):
                    ps = psp.tile([128, 512], F32)
                    for cc in range(CT):
                        nc.tensor.matmul(
                            ps,
                            lhsT=xT[cc][:, ts(tt, 128)],
                            rhs=wv_sb[:, cc, :],
                            start=(cc == 0),
                            stop=(cc == CT - 1),
                        )
                    nc.vector.tensor_copy(
                        out=vS[:, tt, :, 0:64],
                        in_=ps.rearrange("p (h d) -> p h d", h=8),
                    )

                def qkproj_tc(tc_):
                    for p in range(NP):
                        for w_sb, dst, scl in ((wq_sb, qtd, SCALE), (wk_sb, ktd, 1.0)):
                            ps = psp.tile([128, 512], F32)
                            for cc in range(CT):
                                nc.tensor.matmul(
                                    ps,
                                    lhsT=w_sb[:, cc, p, :],
                                    rhs=xT[cc][:, ts(tc_, 512)],
                                    start=(cc == 0),
                                    stop=(cc == CT - 1),
                                )
                            bo = bpool.tile([128, 512], F32)
                            nc.scalar.mul(out=bo, in_=ps, mul=scl)
                            nc.sync.dma_start(out=dst[(p, tc_)], in_=bo)

                for tt in range(TT + 1):
                    if tt < TT:
                        transpose_tt(tt)
                    if tt == 0:
                        load_weights()
                    if tt >= 1:
                        vproj_tt(tt - 1)
                        if (tt - 1) % 4 == 3:
                            qkproj_tc((tt - 1) // 4)

            # ---------------- phase 2: attention ----------------
            # Emission keeps the PE in same-type runs: a group of 3 j-steps of
            # S matmuls (+exp on ACT), then the previous group's AV matmuls.
            # Interleaving S/AV per-j costs ~25%/MM in PE streaming rate.
            with (
                tc.tile_pool(name="qkc", bufs=8) as qkcpool,
                tc.tile_pool(name="ptp", bufs=14) as ptpool,
                tc.tile_pool(name="rsm", bufs=4) as rpool,
                tc.tile_pool(name="rbcp", bufs=4) as rbcpool,
                tc.tile_pool(name="psS", bufs=3, space="PSUM") as psS,
                tc.tile_pool(name="psAv", bufs=2, space="PSUM") as psAv,
            ):
                qch = {}
                kch = {}
                avs = {}
                pts = {}

                def fetch_pair(p):
                    for tc_ in range(4):
                        qt = qkcpool.tile([128, 512], F32R, name="qTc", tag="qTc")
                        nc.sync.dma_start(out=qt, in_=qtd[(p, tc_)].bitcast(F32R))
                        qch[(p, tc_)] = qt
                        kt = qkcpool.tile([128, 512], F32R, name="kTc", tag="kTc")
                        nc.sync.dma_start(out=kt, in_=ktd[(p, tc_)].bitcast(F32R))
                        kch[(p, tc_)] = kt

                def s_exp(p, qc, j):
                    off = max(0, 128 * j - 512 * qc)
                    sg = psS.tile([128, 2, 512], F32)
                    kc = kch[(p, j // 4)]
                    qc_t = qch[(p, qc)]
                    jo = 128 * (j % 4)
                    for m in range(2):
                        nc.tensor.matmul(
                            sg[:, m, off:],
                            lhsT=kc[64 * m : 64 * m + 64, jo : jo + 128],
                            rhs=qc_t[64 * m : 64 * m + 64, off:],
                            start=True,
                            stop=True,
                        )
                    ptile = ptpool.tile([128, 2, 512], F32R)
                    nc.scalar.activation(
                        out=ptile[:, :, off:], in_=sg[:, :, off:], func=EXP
                    )
                    if j >= 4 * qc:
                        nc.vector.tensor_mul(
                            ptile[:, :, off : off + 128],
                            ptile[:, :, off : off + 128],
                            tr[:, None, :].to_broadcast([128, 2, 128]),
                        )
                    pts[(p, qc, j)] = (ptile, off)

                def av_mm(p, qc, j, nj):
                    ptile, off = pts.pop((p, qc, j))
                    av = avs[(p, qc)]
                    for m in range(2):
                        nc.tensor.matmul(
                            av[m][:65, off:],
                            lhsT=vS[:, j, 2 * p + m, :],
                            rhs=ptile[:, m, off:],
                            start=(j == 0),
                            stop=(j == nj - 1),
                        )

                def normalize(p, qc):
                    av = avs.pop((p, qc))
                    rsbs = []
                    for m in range(2):
                        rsb = rpool.tile([1, 512], F32, name="rsb", tag="rsb")
                        nc.vector.tensor_copy(out=rsb, in_=av[m][64:65, :])
                        # unnormalized O~ out of PSUM so the av bank frees fast
                        nc.vector.tensor_copy(
                            out=OT[64 * m : 64 * m + 64, p, ts(qc, 512)],
                            in_=av[m][0:64, :],
                        )
                        rsbs.append(rsb)
                    for m in range(2):
                        rinv = rpool.tile([1, 512], F32, name="rinv", tag="rinv")
                        nc.vector.reciprocal_approx_fast(out=rinv, in_=rsbs[m])
                        rb = rbcpool.tile([128, 512], F32, name="rb", tag="rb")
                        nc.gpsimd.partition_broadcast(rb, rinv)
                        sl = OT[64 * m : 64 * m + 64, p, ts(qc, 512)]
                        nc.vector.tensor_mul(sl, sl, rb[64 * m : 64 * m + 64, :])

                groups = []
                for p in range(NP):
                    for qc in range(NQC):
                        nj = 4 * qc + 4
                        js = list(range(nj))
                        sub = [js[i : i + 3] for i in range(0, nj, 3)]
                        for gi, jg in enumerate(sub):
                            groups.append((p, qc, nj, jg, gi == 0, gi == len(sub) - 1))

                def av_group(gi):
                    p, qc, nj, jg, first, last = groups[gi]
                    if first:
                        avs[(p, qc)] = [
                            psAv.tile([128, 512], F32, name="av", tag="av")
                            for _ in range(2)
                        ]
                    for j in jg:
                        av_mm(p, qc, j, nj)
                    if last:
                        normalize(p, qc)

                # S-runs of 6 MMs; AV-runs of ~12 (two groups) to amortize the
                # PE row-config switch between K=64 S and K=128 AV matmuls
                LAG = 2
                for i in range(len(groups) + LAG):
                    if i < len(groups):
                        p, qc, nj, jg, first, last = groups[i]
                        if qc == 0 and first:
                            fetch_pair(p)
                        for j in jg:
                            s_exp(p, qc, j)
                    if i >= LAG and (i - LAG) % 2 == 1:
                        av_group(i - LAG - 1)
                        av_group(i - LAG)
                if len(groups) % 2 == 1:
                    av_group(len(groups) - 1)

            # ---------------- phase 3: output projection ----------------
            with (
                tc.tile_pool(name="obnc", bufs=4) as opool,
                tc.tile_pool(name="psO", bufs=8, space="PSUM") as psO,
            ):
                for ft in range(8):
                    pso = [psO.tile([128, 512], F32, name="pso", tag="pso") for _ in range(4)]
                    for p in range(NP):
                        for tc_ in range(4):
                            nc.tensor.matmul(
                                pso[tc_],
                                lhsT=wo_sb[:, p, ts(ft, 128)],
                                rhs=OT[:, p, ts(tc_, 512)],
                                start=(p == 0),
                                stop=(p == NP - 1),
                            )
                    for tc_ in range(4):
                        ob = opool.tile([128, 512], F32)
                        nc.vector.tensor_copy(out=ob, in_=pso[tc_])
                        nc.sync.dma_start(out=ot[ts(ft, 128), ts(tc_, 512)], in_=ob)

    nc.compile()
    return nc


def kernel(x, W_qkv, b_qkv, W_out, b_out):
    global _last_in_maps
    x = np.asarray(x, dtype=np.float32)
    W_qkv = np.asarray(W_qkv, dtype=np.float32)
    b_qkv = np.asarray(b_qkv, dtype=np.float32)
    W_out = np.asarray(W_out, dtype=np.float32)
    b_out = np.asarray(b_out, dtype=np.float32)
    B = x.shape[0]

    aug = bool(np.any(b_qkv))
    CT = 9 if aug else 8
    if CT not in _CACHE:
        _CACHE[CT] = _build(CT)
    nc = _CACHE[CT]

    # triangle keep-mask for the diagonal 128 block: [p, c] = 1 if c >= p
    tri = (np.arange(128)[None, :] >= np.arange(128)[:, None]).astype(np.float32)

    in_maps = []
    for core in range(8):
        b, g = core // 2, core % 2
        xa = x[b]
        if aug:
            pad = np.zeros((T, 128), np.float32)
            pad[:, 0] = 1.0
            xa = np.concatenate([xa, pad], axis=1)

        def wslice(col0):
            w = W_qkv[:, col0 + 512 * g : col0 + 512 * g + 512]
            if aug:
                extra = np.zeros((128, 512), np.float32)
                extra[0] = b_qkv[col0 + 512 * g : col0 + 512 * g + 512]
                w = np.concatenate([w, extra], axis=0)
            return np.ascontiguousarray(w)

        in_maps.append(
            {
                "xa": np.ascontiguousarray(xa),
                "wq": wslice(0),
                "wk": wslice(1024),
                "wv": wslice(2048),
                "wo": np.ascontiguousarray(W_out[512 * g : 512 * g + 512, :]),
                "tri": tri,
            }
        )

    _last_in_maps = in_maps
    res = bass_utils.run_bass_kernel_spmd(nc, in_maps, list(range(8))).results
    out = np.empty((B, T, 1024), np.float32)
    for b in range(B):
        acc = res[2 * b]["ot"] + res[2 * b + 1]["ot"]
        out[b] = acc.T + b_out[None, :]
    return out
